# revision 2
# baseline (speedup 1.0000x reference)
"""Trainium2 Bass kernel for channel-wise spatial attention.

Reference computation (B=4, C=64, S=96, H=8):
  vqk = 1x1conv(x, w_vkq) + b_vkq            -> (B, 3*H*C, S, S)
  per (b,h,c):  score[r,t] = sum_y v[r,y]*k[t,y] / S^2 ; s = softmax_t
                out2[r,t]  = sum_y s[r,y]*q[t,y]
  out = 1x1conv(rearrange(out2, 'b h c x z -> b (c h) x z'), w_out) + b_out

Sharding: 8 cores = 4 batches x 2 head-halves (4 heads each). Each core
computes a partial to_out projection over its 256 (c,h) pairs; host sums
the two partials per batch and adds b_out.

Layout: the vkq projection uses the x-spatial-slice of the input as the
stationary matmul operand (lhsT = [x[b][:, xslice]; ones] of shape
[65, 96], bias folded in via the ones row), so projection outputs land as
[second-spatial-on-partitions, channel] psum tiles. The psum->SBUF cast
scatters them into a CHANNEL-MAJOR layout: per half-head group, three
sections [V | Q | K], each 32 slots of 97 columns (96 data cols = first
spatial index, 97th col = softmax-ones for Q / dead padding for V,K).
That makes every attention operand contiguous:
  mm1: score_T = Kwin^T @ V_c  with Kwin a 128-column stationary window
       (96 real + 32 spill) -> FWL-eligible weight loads (the LDWEIGHTS
       cadence, 1.67 ns/col without FWL, is the attention bottleneck)
  exp via ACT; mm2: out2 = Ewin^T @ [Q_c | ones] (denominator fused).
The (c,h) gather for to_out bounces through DRAM (contiguous write,
strided read) in 2 chunks per group; to_out is K=128 PSUM-accumulated
matmuls over 512-pixel chunks.

All matmul operands are bf16 (PSUM stays fp32). ACT runs Exp + half the
psum->SBUF casts; DVE runs the other casts, reciprocals, normalizes.
"""

import os
import sys
from contextlib import ExitStack

sys.path.insert(0, "/opt/trn_rl_repo")

import numpy as np

import concourse.bacc as bacc
import concourse.tile as tile
from concourse import mybir
from concourse.bass_utils import run_bass_kernel_spmd

B, C, S, H = 4, 64, 96, 8
NPIX = S * S
HL = H // 2      # heads per core
NQ = 8           # half-head groups per core
CL = 32          # attention channels per group
JW = 3 * CL      # projection channels per group (v,q,k)
QP = 4           # half-head groups per projection pass
NCORES = 8
FCH = 512        # final projection free-dim chunk
CB = 4           # attention channels batched per exp/normalize op

SLOT = S + 1             # 97: per-channel column slot (96 data + 1 extra)
SECW = CL * SLOT         # section width (V, Q or K) per group
GRPW = 3 * SECW          # group width
PASSW = QP * GRPW        # projection pass width
PAD = 32                 # tail pad so 128-col stationary windows stay in-bounds

F32 = mybir.dt.float32
BF16 = mybir.dt.bfloat16

# mmdt: projection/attention matmul operands; findt: gathered out2 + to_out
# mmw: attention stationary window width (128 -> FWL weight loads)
CFG = {"mmdt": BF16, "findt": BF16, "mmw": 128}


def _body(ctx, tc, xe, wtg, w2t, outp, cfg):
    nc = tc.nc
    mmdt = cfg["mmdt"]
    findt = cfg["findt"]
    MMW = cfg["mmw"]
    Exp = mybir.ActivationFunctionType.Exp
    Copy = mybir.ActivationFunctionType.Copy

    const = ctx.enter_context(tc.tile_pool(name="const", bufs=1))
    projp = ctx.enter_context(tc.tile_pool(name="projp", bufs=1))
    obp = ctx.enter_context(tc.tile_pool(name="obp", bufs=6))
    etp = ctx.enter_context(tc.tile_pool(name="etp", bufs=3))
    rcp = ctx.enter_context(tc.tile_pool(name="rcp", bufs=3))
    stp = ctx.enter_context(tc.tile_pool(name="stp", bufs=3))
    pp_pool = ctx.enter_context(tc.tile_pool(name="pp", bufs=3, space="PSUM"))
    ps_pool = ctx.enter_context(tc.tile_pool(name="ps", bufs=2, space="PSUM"))
    po_pool = ctx.enter_context(tc.tile_pool(name="po", bufs=3, space="PSUM"))
    pf_pool = ps_pool  # final projection reuses the score psum slots
    dramp = ctx.enter_context(tc.tile_pool(name="dstage", bufs=4, space="DRAM"))

    # weights first (every projection matmul streams them), then the input
    # in 4 chunks so the first projection matmuls start early
    WTG = const.tile([C + 1, NQ * JW], mmdt)
    nc.sync.dma_start(WTG[:], wtg[:])
    XCH = 4
    XW = NPIX // XCH
    XEC = [const.tile([C + 1, XW], mmdt, name=f"xe{i}", tag=f"xe{i}") for i in range(XCH)]
    for i in range(XCH):
        eng = nc.sync if i % 2 == 0 else nc.gpsimd
        eng.dma_start(XEC[i][:], xe[:, i * XW : (i + 1) * XW])

    def xe_slice(x):
        i, r = divmod(x * S, XW)
        return XEC[i][:, r : r + S]

    W2T = const.tile([128, 2 * C], findt)
    nc.gpsimd.dma_start(W2T[:, 0:C], w2t[0:128, :])
    nc.gpsimd.dma_start(W2T[:, C : 2 * C], w2t[128:256, :])
    HFIN = const.tile([128, 2 * NPIX], findt)

    from concourse.bass import broadcast_tensor_aps

    JQ = QP * JW  # projection channels per pass

    for hp in range(NQ // QP):  # projection pass over QP half-head groups
        # Channel-major pass tile: PROJ4[y, g*GRPW + sec*SECW + c*SLOT + x]
        # sec 0=V, 1=Q (97th col = ones), 2=K (97th col = dead).
        PROJ4 = projp.tile([S, PASSW + PAD], mmdt, tag="proj")
        pview = PROJ4[:, 0:PASSW].rearrange(
            "p (g s c x) -> p g s c x", s=3, c=CL, x=SLOT
        )
        nc.gpsimd.memset(pview[:, :, 1, :, S], 1.0)   # softmax-denominator ones
        nc.gpsimd.memset(pview[:, :, 0, :, S], 0.0)   # dead cols (V)
        nc.gpsimd.memset(pview[:, :, 2, :, S], 0.0)   # dead cols (K)
        nc.gpsimd.memset(PROJ4[:, PASSW:], 0.0)       # tail pad

        for x in range(S):
            pp = pp_pool.tile([S, JQ], F32, tag="pp")
            nc.tensor.matmul(
                pp[:],
                lhsT=xe_slice(x),
                rhs=WTG[:, hp * JQ : (hp + 1) * JQ],
                start=True,
                stop=True,
            )
            dst = pview[:, :, :, :, x]
            src = pp[:].rearrange("p (g s c) -> p g s c", s=3, c=CL)
            if x % 2 == 0:
                nc.vector.tensor_copy(dst, src)
            else:
                nc.scalar.activation(dst, src, Copy)

        def vslot(g, c):
            o = g * GRPW + c * SLOT
            return PROJ4[:, o : o + S]

        def qslot97(g, c):
            o = g * GRPW + SECW + c * SLOT
            return PROJ4[:, o : o + SLOT]

        def kwin(g, c):
            o = g * GRPW + 2 * SECW + c * SLOT
            return PROJ4[:, o : o + MMW]

        for ql in range(QP):
            q = hp * QP + ql
            OB = obp.tile([S, CL * S], findt, tag="ob")
            for cl0 in range(0, CL, CB):
                ps4 = ps_pool.tile([MMW, CB * S], F32, tag="ps")
                for i in range(CB):
                    nc.tensor.matmul(
                        ps4[:, i * S : (i + 1) * S],
                        lhsT=kwin(ql, cl0 + i),
                        rhs=vslot(ql, cl0 + i),
                        start=True,
                        stop=True,
                    )
                et4 = etp.tile([S, CB * S + PAD], mmdt, tag="et")
                nc.gpsimd.memset(et4[:, CB * S :], 0.0)
                nc.scalar.activation(
                    et4[:, 0 : CB * S], ps4[0:S, :], Exp, scale=1.0 / NPIX
                )
                po4 = po_pool.tile([MMW, CB * SLOT], F32, tag="po")
                for i in range(CB):
                    nc.tensor.matmul(
                        po4[:, i * SLOT : (i + 1) * SLOT],
                        lhsT=et4[:, i * S : i * S + MMW],
                        rhs=qslot97(ql, cl0 + i),
                        start=True,
                        stop=True,
                    )
                po4v = po4[0:S, :].rearrange("p (i w) -> p i w", w=SLOT)
                rc4 = rcp.tile([S, CB], F32, tag="rc")
                nc.vector.reciprocal(rc4[:], po4v[:, :, S])
                obv = OB[:, cl0 * S : (cl0 + CB) * S].rearrange(
                    "p (i z) -> p i z", z=S
                )
                rc4b, _ = broadcast_tensor_aps(
                    rc4[:].rearrange("p (i o) -> p i o", o=1), po4v[:, :, 0:S]
                )
                nc.vector.tensor_mul(obv, po4v[:, :, 0:S], rc4b)

            # gather: OB[x, (cl, z)] -> HFIN[rows, (x, z)] via a DRAM
            # bounce (contiguous write, strided read with the (c x z)
            # ordering on the DRAM side). Two chunks per group so the
            # final group's gather overlaps its own tail of attention.
            HCL = CL // 2
            for half in range(2):
                r0 = q * CL + half * HCL
                hfh, row = divmod(r0, 128)
                DQ = dramp.tile([S, HCL * S], findt, tag="dq")
                nc.sync.dma_start(
                    DQ[:], OB[:, half * HCL * S : (half + 1) * HCL * S]
                )
                nc.gpsimd.dma_start(
                    HFIN[row : row + HCL, hfh * NPIX : (hfh + 1) * NPIX],
                    DQ[:].rearrange("x (c z) -> c x z", z=S),
                )

    # to_out projection: contract all 256 (h,c) rows
    for n0 in range(0, NPIX, FCH):
        pf = pf_pool.tile([C, FCH], F32, tag="ps")
        nc.tensor.matmul(
            pf[:], lhsT=W2T[:, 0:C], rhs=HFIN[:, n0 : n0 + FCH], start=True, stop=False
        )
        nc.tensor.matmul(
            pf[:],
            lhsT=W2T[:, C : 2 * C],
            rhs=HFIN[:, NPIX + n0 : NPIX + n0 + FCH],
            start=False,
            stop=True,
        )
        fst = stp.tile([C, FCH], F32, tag="fst")
        if (n0 // FCH) % 2 == 0:
            nc.vector.tensor_copy(fst[:], pf[:])
        else:
            nc.scalar.activation(fst[:], pf[:], Copy)
        del pf
        nc.sync.dma_start(outp[:, n0 : n0 + FCH], fst[:])


_NC_CACHE = {}


def build_nc(cfg_key=None):
    cfg = CFG
    key = (cfg["mmdt"], cfg["findt"], cfg["mmw"])
    if key in _NC_CACHE:
        return _NC_CACHE[key]
    nc = bacc.Bacc("TRN2", target_bir_lowering=False, debug=False)
    xe = nc.dram_tensor("xe", [C + 1, NPIX], cfg["mmdt"], kind="ExternalInput").ap()
    wtg = nc.dram_tensor(
        "wtg", [C + 1, NQ * JW], cfg["mmdt"], kind="ExternalInput"
    ).ap()
    w2t = nc.dram_tensor("w2t", [2 * 128, C], cfg["findt"], kind="ExternalInput").ap()
    outp = nc.dram_tensor("outp", [C, NPIX], F32, kind="ExternalOutput").ap()
    with tile.TileContext(nc) as tc:
        with ExitStack() as ctx:
            _body(ctx, tc, xe, wtg, w2t, outp, cfg)
    nc.compile()
    _NC_CACHE[key] = nc
    return nc


def prep_in_maps(x, w_vkq, b_vkq, w_out, b_out):
    mmnp = np.dtype(mybir.dt.np(CFG["mmdt"]))
    finp = np.dtype(mybir.dt.np(CFG["findt"]))
    x = np.asarray(x, np.float32)
    w_vkq = np.asarray(w_vkq, np.float32)
    b_vkq = np.asarray(b_vkq, np.float32)
    w_out = np.asarray(w_out, np.float32)
    in_maps = []
    for core in range(NCORES):
        b, hh = divmod(core, 2)
        xe = np.concatenate(
            [x[b].reshape(C, NPIX), np.ones((1, NPIX), np.float32)], axis=0
        )
        wtg = np.empty((C + 1, NQ * JW), np.float32)
        w2t = np.empty((256, C), np.float32)
        for qq in range(NQ):
            h = hh * HL + qq // 2
            cb = (qq % 2) * CL
            for s in range(3):
                o = s * (H * C) + h * C + cb
                j = qq * JW + s * CL
                wtg[0:C, j : j + CL] = w_vkq[o : o + CL, :].T
                wtg[C, j : j + CL] = b_vkq[o : o + CL]
            for cl in range(CL):
                w2t[qq * CL + cl, :] = w_out[:, (cb + cl) * H + h]
        in_maps.append(
            {
                "xe": xe.astype(mmnp),
                "wtg": wtg.astype(mmnp),
                "w2t": w2t.astype(finp),
            }
        )
    return in_maps


def combine(results, b_out):
    b_out = np.asarray(b_out, np.float32)
    out = np.empty((B, C, S, S), np.float32)
    for b in range(B):
        part = results[2 * b]["outp"].astype(np.float32) + results[2 * b + 1][
            "outp"
        ].astype(np.float32)
        out[b] = part.reshape(C, S, S) + b_out[:, None, None]
    return out


def kernel(x, w_vkq, b_vkq, w_out, b_out):
    nc = build_nc()
    in_maps = prep_in_maps(x, w_vkq, b_vkq, w_out, b_out)
    r = run_bass_kernel_spmd(nc, in_maps, list(range(NCORES)), trace=False)
    kernel.last_result = r
    return combine(r.results, b_out)


# revision 4
# speedup vs baseline: 1.1464x; 1.1464x over previous
"""Trainium2 Bass kernel for channel-wise spatial attention.

Reference computation (B=4, C=64, S=96, H=8):
  vqk = 1x1conv(x, w_vkq) + b_vkq            -> (B, 3*H*C, S, S)
  per (b,h,c):  score[r,t] = sum_y v[r,y]*k[t,y] / S^2 ; s = softmax_t
                out2[r,t]  = sum_y s[r,y]*q[t,y]
  out = 1x1conv(rearrange(out2, 'b h c x z -> b (c h) x z'), w_out) + b_out

Sharding: 8 cores = 4 batches x 2 head-halves (4 heads each). Each core
computes a partial to_out projection over its 256 (c,h) pairs; host sums
the two partials per batch and adds b_out.

Layout trick: the vkq projection uses the x-spatial-slice of the input as
the *stationary* matmul operand (lhsT = [x[b][:, xslice]; ones] of shape
[65, 96], bias folded in via the ones row), so projection outputs land as
[second-spatial-on-partitions, channel] tiles, psum->SBUF cast writes are
contiguous, and every attention matmul is transpose-free:
  mm1: score_T = K_slice^T @ V_slice           (psum [z, x])
  exp via ACT (scores ~1e-5, no max subtraction needed)
  mm2: out2 = E_T^T @ Q_slice with a 128-column E_T stationary window
       (96 real + 32 zero pad): a full-width weight load engages the
       PE's Fast Weight Load path (~66 ns vs 163 ns for 96 columns),
       which is the attention-phase rate limiter. The rhs spans 97
       columns (x-slot S holds ones) so one matmul also yields the
       softmax denominator.
The (c,h) gather for to_out bounces through DRAM (contiguous write,
strided read) in 2 chunks per group; to_out is K=128 PSUM-accumulated
matmuls. Input DMAs are split weights-first across three dispatch
engines so the first projection matmul starts ~2 us in.
"""

import os
import sys
from contextlib import ExitStack

sys.path.insert(0, "/opt/trn_rl_repo")

import numpy as np

import concourse.bacc as bacc
import concourse.tile as tile
from concourse import mybir
from concourse.bass_utils import run_bass_kernel_spmd

B, C, S, H = 4, 64, 96, 8
NPIX = S * S
HL = H // 2      # heads per core
NQ = 8           # half-head groups per core
CL = 32          # attention channels per group
JW = 3 * CL      # projection channels per group (v,q,k)
QP = 4           # half-head groups per projection pass
NCORES = 8
FCH = 512        # final projection free-dim chunk
CB = 4           # attention channels batched per exp/normalize op
MMW = 128        # mm2 stationary window width (128 -> FWL weight loads)
PAD = MMW - S    # et4 tail pad so the last window stays in-bounds

F32 = mybir.dt.float32
BF16 = mybir.dt.bfloat16

# mmdt: projection/attention matmul operands; findt: gathered out2 + to_out
CFG = {"mmdt": BF16, "findt": BF16}


def _body(ctx, tc, xe, wtg, w2t, outp, cfg):
    nc = tc.nc
    mmdt = cfg["mmdt"]
    findt = cfg["findt"]
    Exp = mybir.ActivationFunctionType.Exp
    Copy = mybir.ActivationFunctionType.Copy

    const = ctx.enter_context(tc.tile_pool(name="const", bufs=1))
    projp = ctx.enter_context(tc.tile_pool(name="projp", bufs=1))
    obp = ctx.enter_context(tc.tile_pool(name="obp", bufs=6))
    etp = ctx.enter_context(tc.tile_pool(name="etp", bufs=3))
    rcp = ctx.enter_context(tc.tile_pool(name="rcp", bufs=3))
    stp = ctx.enter_context(tc.tile_pool(name="stp", bufs=3))
    pp_pool = ctx.enter_context(tc.tile_pool(name="pp", bufs=4, space="PSUM"))
    ps_pool = ctx.enter_context(tc.tile_pool(name="ps", bufs=2, space="PSUM"))
    po_pool = ctx.enter_context(tc.tile_pool(name="po", bufs=2, space="PSUM"))
    pf_pool = ps_pool  # final projection reuses the score psum slots
    dramp = ctx.enter_context(tc.tile_pool(name="dstage", bufs=4, space="DRAM"))

    JQ = QP * JW  # projection channels per pass

    # input loads: first projection pass needs wtg cols [0:JQ] and the
    # leading x columns, so those go first on the two HWDGE dispatch
    # engines; everything else trails on the gpsimd (SWDGE) queue.
    WTG = const.tile([C + 1, NQ * JW], mmdt)
    nc.sync.dma_start(WTG[:, 0:JQ], wtg[:, 0:JQ])
    XSPLIT = [0, 384, 3360, 6336, NPIX]
    XEC = []
    for i in range(4):
        lo, hi = XSPLIT[i], XSPLIT[i + 1]
        t = const.tile([C + 1, hi - lo], mmdt, name=f"xe{i}", tag=f"xe{i}")
        XEC.append(t)
        eng = nc.scalar if i == 0 else nc.gpsimd
        eng.dma_start(t[:], xe[:, lo:hi])
    nc.gpsimd.dma_start(WTG[:, JQ : 2 * JQ], wtg[:, JQ : 2 * JQ])

    def xe_slice(x):
        p = x * S
        for i in range(4):
            if p < XSPLIT[i + 1]:
                r = p - XSPLIT[i]
                return XEC[i][:, r : r + S]

    W2T = const.tile([128, 2 * C], findt)
    nc.gpsimd.dma_start(W2T[:, 0:C], w2t[0:128, :])
    nc.gpsimd.dma_start(W2T[:, C : 2 * C], w2t[128:256, :])
    HFIN = const.tile([128, 2 * NPIX], findt)

    from concourse.bass import broadcast_tensor_aps

    for hp in range(NQ // QP):  # projection pass over QP half-head groups
        # PROJ4[y, (x, jj)] = vqk_raw[b, o(hp*QP + jj//JW, jj%JW), x, y]
        # x-major so the psum->SBUF cast writes contiguously. x-slot S
        # holds ones: mm2's rhs spans 97 columns so one matmul yields
        # out2 plus the softmax denominator.
        PROJ4 = projp.tile([S, (S + 1) * JQ], mmdt, tag="proj")
        projv = PROJ4[:].rearrange("p (x jj) -> p x jj", jj=JQ)
        nc.gpsimd.memset(PROJ4[:, S * JQ : (S + 1) * JQ], 1.0)
        for x in range(S):
            pp = pp_pool.tile([S, JQ], F32, tag="pp")
            nc.tensor.matmul(
                pp[:],
                lhsT=xe_slice(x),
                rhs=WTG[:, hp * JQ : (hp + 1) * JQ],
                start=True,
                stop=True,
            )
            dst = PROJ4[:, x * JQ : (x + 1) * JQ]
            if x % 2 == 0:
                nc.vector.tensor_copy(dst, pp[:])
            else:
                nc.scalar.activation(dst, pp[:], Copy)

        for ql in range(QP):
            q = hp * QP + ql
            OB = obp.tile([S, CL * S], findt, tag="ob")
            for cl0 in range(0, CL, CB):
                ps4 = ps_pool.tile([S, CB * S], F32, tag="ps")
                for i in range(CB):
                    cl = cl0 + i
                    vsl = projv[:, 0:S, ql * JW + 0 * CL + cl]
                    ksl = projv[:, 0:S, ql * JW + 2 * CL + cl]
                    nc.tensor.matmul(
                        ps4[:, i * S : (i + 1) * S],
                        lhsT=ksl,
                        rhs=vsl,
                        start=True,
                        stop=True,
                    )
                et4 = etp.tile([S, CB * S + PAD], mmdt, tag="et")
                nc.gpsimd.memset(et4[:, CB * S :], 0.0)
                nc.scalar.activation(
                    et4[:, 0 : CB * S], ps4[:], Exp, scale=1.0 / NPIX
                )
                po4 = po_pool.tile([MMW, CB * (S + 1)], F32, tag="po")
                for i in range(CB):
                    cl = cl0 + i
                    q97 = projv[:, :, ql * JW + 1 * CL + cl]
                    nc.tensor.matmul(
                        po4[:, i * (S + 1) : (i + 1) * (S + 1)],
                        lhsT=et4[:, i * S : i * S + MMW],
                        rhs=q97,
                        start=True,
                        stop=True,
                    )
                po4v = po4[0:S, :].rearrange("p (i w) -> p i w", w=S + 1)
                rc4 = rcp.tile([S, CB], F32, tag="rc")
                nc.vector.reciprocal(rc4[:], po4v[:, :, S])
                obv = OB[:, cl0 * S : (cl0 + CB) * S].rearrange(
                    "p (i z) -> p i z", z=S
                )
                rc4b, _ = broadcast_tensor_aps(
                    rc4[:].rearrange("p (i o) -> p i o", o=1), po4v[:, :, 0:S]
                )
                nc.vector.tensor_mul(obv, po4v[:, :, 0:S], rc4b)

            # gather: OB[x, (cl, z)] -> HFIN[rows, (x, z)] via a DRAM
            # bounce (contiguous write, then strided read with the
            # (c x z) ordering on the DRAM side). Two chunks per group
            # so the final group's gather overlaps its own attention.
            HCL = CL // 2
            for half in range(2):
                r0 = q * CL + half * HCL
                hfh, row = divmod(r0, 128)
                DQ = dramp.tile([S, HCL * S], findt, tag="dq")
                nc.sync.dma_start(
                    DQ[:], OB[:, half * HCL * S : (half + 1) * HCL * S]
                )
                nc.gpsimd.dma_start(
                    HFIN[row : row + HCL, hfh * NPIX : (hfh + 1) * NPIX],
                    DQ[:].rearrange("x (c z) -> c x z", z=S),
                )

    # to_out projection: contract all 256 (h,c) rows
    for n0 in range(0, NPIX, FCH):
        pf = pf_pool.tile([C, FCH], F32, tag="ps")
        nc.tensor.matmul(
            pf[:], lhsT=W2T[:, 0:C], rhs=HFIN[:, n0 : n0 + FCH], start=True, stop=False
        )
        nc.tensor.matmul(
            pf[:],
            lhsT=W2T[:, C : 2 * C],
            rhs=HFIN[:, NPIX + n0 : NPIX + n0 + FCH],
            start=False,
            stop=True,
        )
        fst = stp.tile([C, FCH], F32, tag="fst")
        if (n0 // FCH) % 2 == 0:
            nc.vector.tensor_copy(fst[:], pf[:])
        else:
            nc.scalar.activation(fst[:], pf[:], Copy)
        del pf
        nc.sync.dma_start(outp[:, n0 : n0 + FCH], fst[:])


_NC_CACHE = {}


def build_nc(cfg_key=None):
    cfg = CFG
    key = (cfg["mmdt"], cfg["findt"])
    if key in _NC_CACHE:
        return _NC_CACHE[key]
    nc = bacc.Bacc("TRN2", target_bir_lowering=False, debug=False)
    xe = nc.dram_tensor("xe", [C + 1, NPIX], cfg["mmdt"], kind="ExternalInput").ap()
    wtg = nc.dram_tensor(
        "wtg", [C + 1, NQ * JW], cfg["mmdt"], kind="ExternalInput"
    ).ap()
    w2t = nc.dram_tensor("w2t", [2 * 128, C], cfg["findt"], kind="ExternalInput").ap()
    outp = nc.dram_tensor("outp", [C, NPIX], F32, kind="ExternalOutput").ap()
    with tile.TileContext(nc) as tc:
        with ExitStack() as ctx:
            _body(ctx, tc, xe, wtg, w2t, outp, cfg)
    nc.compile()
    _NC_CACHE[key] = nc
    return nc


def prep_in_maps(x, w_vkq, b_vkq, w_out, b_out):
    mmnp = np.dtype(mybir.dt.np(CFG["mmdt"]))
    finp = np.dtype(mybir.dt.np(CFG["findt"]))
    x = np.asarray(x, np.float32)
    w_vkq = np.asarray(w_vkq, np.float32)
    b_vkq = np.asarray(b_vkq, np.float32)
    w_out = np.asarray(w_out, np.float32)
    in_maps = []
    for core in range(NCORES):
        b, hh = divmod(core, 2)
        xe = np.concatenate(
            [x[b].reshape(C, NPIX), np.ones((1, NPIX), np.float32)], axis=0
        )
        wtg = np.empty((C + 1, NQ * JW), np.float32)
        w2t = np.empty((256, C), np.float32)
        for qq in range(NQ):
            h = hh * HL + qq // 2
            cb = (qq % 2) * CL
            for s in range(3):
                o = s * (H * C) + h * C + cb
                j = qq * JW + s * CL
                wtg[0:C, j : j + CL] = w_vkq[o : o + CL, :].T
                wtg[C, j : j + CL] = b_vkq[o : o + CL]
            for cl in range(CL):
                w2t[qq * CL + cl, :] = w_out[:, (cb + cl) * H + h]
        in_maps.append(
            {
                "xe": xe.astype(mmnp),
                "wtg": wtg.astype(mmnp),
                "w2t": w2t.astype(finp),
            }
        )
    return in_maps


def combine(results, b_out):
    b_out = np.asarray(b_out, np.float32)
    out = np.empty((B, C, S, S), np.float32)
    for b in range(B):
        part = results[2 * b]["outp"].astype(np.float32) + results[2 * b + 1][
            "outp"
        ].astype(np.float32)
        out[b] = part.reshape(C, S, S) + b_out[:, None, None]
    return out


def kernel(x, w_vkq, b_vkq, w_out, b_out):
    nc = build_nc()
    in_maps = prep_in_maps(x, w_vkq, b_vkq, w_out, b_out)
    r = run_bass_kernel_spmd(nc, in_maps, list(range(NCORES)), trace=False)
    kernel.last_result = r
    return combine(r.results, b_out)


# revision 5
# speedup vs baseline: 1.2572x; 1.0966x over previous
"""Trainium2 Bass kernel for channel-wise spatial attention.

Reference computation (B=4, C=64, S=96, H=8):
  vqk = 1x1conv(x, w_vkq) + b_vkq            -> (B, 3*H*C, S, S)
  per (b,h,c):  score[r,t] = sum_y v[r,y]*k[t,y] / S^2 ; s = softmax_t
                out2[r,t]  = sum_y s[r,y]*q[t,y]
  out = 1x1conv(rearrange(out2, 'b h c x z -> b (c h) x z'), w_out) + b_out

Sharding: 8 cores = 4 batches x 2 head-halves (4 heads each). Each core
computes a partial to_out projection over its 256 (c,h) pairs; host sums
the two partials per batch and adds b_out.

Layout trick: the vkq projection uses the x-spatial-slice of the input as
the *stationary* matmul operand (lhsT = [x[b][:, xslice]; ones] of shape
[65, 96], bias folded in via the ones row), so projection outputs land as
[second-spatial-on-partitions, channel] tiles, the psum->SBUF casts write
contiguously, and every attention matmul is transpose-free:
  mm1: score_T = K_slice^T @ V_slice           (psum [z, x])
  exp via ACT (scores ~1e-5, no max subtraction needed)
  mm2: out2 = E_T^T @ Q_slice, rhs spans 97 columns (x-slot S holds ones)
  so one matmul also yields the softmax denominator.

Pipelining: the projection runs in 4 passes of 2 half-head groups with a
double-buffered pass tile, so pass k's projection matmuls + psum casts
overlap pass k-1's attention instead of serializing on a single buffer
(the psum->SBUF cast rate, ~320 ns per x-slice on two engines, is the
projection wall; attention hides it). The (c,h) gather for to_out
bounces through DRAM in 2 chunks per group; to_out is K=128
PSUM-accumulated matmuls. Input DMAs go weights-first on the HWDGE
queues so the first matmul starts ~2 us in.
"""

import os
import sys
from contextlib import ExitStack

sys.path.insert(0, "/opt/trn_rl_repo")

import numpy as np

import concourse.bacc as bacc
import concourse.tile as tile
from concourse import mybir
from concourse.bass_utils import run_bass_kernel_spmd

B, C, S, H = 4, 64, 96, 8
NPIX = S * S
HL = H // 2      # heads per core
NQ = 8           # half-head groups per core
CL = 32          # attention channels per group
JW = 3 * CL      # projection channels per group (v,q,k)
QP = 2           # half-head groups per projection pass
NCORES = 8
FCH = 512        # final projection free-dim chunk
CB = 4           # attention channels batched per exp/normalize op

F32 = mybir.dt.float32
BF16 = mybir.dt.bfloat16

# mmdt: projection/attention matmul operands; findt: gathered out2 + to_out
CFG = {"mmdt": BF16, "findt": BF16}


def _body(ctx, tc, xe, wtg, w2t, outp, cfg):
    nc = tc.nc
    mmdt = cfg["mmdt"]
    findt = cfg["findt"]
    Exp = mybir.ActivationFunctionType.Exp
    Copy = mybir.ActivationFunctionType.Copy

    const = ctx.enter_context(tc.tile_pool(name="const", bufs=1))
    projp = ctx.enter_context(tc.tile_pool(name="projp", bufs=2))
    obp = ctx.enter_context(tc.tile_pool(name="obp", bufs=4))
    etp = ctx.enter_context(tc.tile_pool(name="etp", bufs=3))
    rcp = ctx.enter_context(tc.tile_pool(name="rcp", bufs=3))
    stp = ctx.enter_context(tc.tile_pool(name="stp", bufs=3))
    pp_pool = ctx.enter_context(tc.tile_pool(name="pp", bufs=3, space="PSUM"))
    ps_pool = ctx.enter_context(tc.tile_pool(name="ps", bufs=2, space="PSUM"))
    po_pool = ctx.enter_context(tc.tile_pool(name="po", bufs=3, space="PSUM"))
    pf_pool = ps_pool  # final projection reuses the score psum slots
    dramp = ctx.enter_context(tc.tile_pool(name="dstage", bufs=4, space="DRAM"))

    JQ = QP * JW  # projection channels per pass (192)

    # input loads, weights-first on the HWDGE queue: the first pass needs
    # wtg cols [0:JQ] and the leading x columns only
    WTG = const.tile([C + 1, NQ * JW], mmdt)
    nc.sync.dma_start(WTG[:, 0:JQ], wtg[:, 0:JQ])
    XSPLIT = [0, 384, 3360, 6336, NPIX]
    XEC = []
    for i in range(4):
        lo, hi = XSPLIT[i], XSPLIT[i + 1]
        t = const.tile([C + 1, hi - lo], mmdt, name=f"xe{i}", tag=f"xe{i}")
        XEC.append(t)
        nc.sync.dma_start(t[:], xe[:, lo:hi])
        if i == 0:
            nc.sync.dma_start(WTG[:, JQ:], wtg[:, JQ:])

    def xe_slice(x):
        p = x * S
        for i in range(4):
            if p < XSPLIT[i + 1]:
                r = p - XSPLIT[i]
                return XEC[i][:, r : r + S]

    W2T = const.tile([128, 2 * C], findt)
    nc.sync.dma_start(W2T[:, 0:C], w2t[0:128, :])
    nc.sync.dma_start(W2T[:, C : 2 * C], w2t[128:256, :])
    HFIN = const.tile([128, 2 * NPIX], findt)

    from concourse.bass import broadcast_tensor_aps

    for hp in range(NQ // QP):  # projection pass over QP half-head groups
        # PROJ4[y, (x, jj)] = vqk_raw[b, o(hp*QP + jj//JW, jj%JW), x, y]
        # x-major so the psum->SBUF cast writes contiguously. x-slot S
        # holds ones: mm2's rhs spans 97 columns so one matmul yields
        # out2 plus the softmax denominator.
        PROJ4 = projp.tile([S, (S + 1) * JQ], mmdt, tag="proj")
        projv = PROJ4[:].rearrange("p (x jj) -> p x jj", jj=JQ)
        nc.gpsimd.memset(PROJ4[:, S * JQ : (S + 1) * JQ], 1.0)
        for x in range(S):
            pp = pp_pool.tile([S, JQ], F32, tag="pp")
            nc.tensor.matmul(
                pp[:],
                lhsT=xe_slice(x),
                rhs=WTG[:, hp * JQ : (hp + 1) * JQ],
                start=True,
                stop=True,
            )
            dst = PROJ4[:, x * JQ : (x + 1) * JQ]
            if x % 2 == 0:
                nc.vector.tensor_copy(dst, pp[:])
            else:
                nc.scalar.activation(dst, pp[:], Copy)

        for ql in range(QP):
            q = hp * QP + ql
            OB = obp.tile([S, CL * S], findt, tag="ob")
            for cl0 in range(0, CL, CB):
                ps4 = ps_pool.tile([S, CB * S], F32, tag="ps")
                for i in range(CB):
                    cl = cl0 + i
                    vsl = projv[:, 0:S, ql * JW + 0 * CL + cl]
                    ksl = projv[:, 0:S, ql * JW + 2 * CL + cl]
                    nc.tensor.matmul(
                        ps4[:, i * S : (i + 1) * S],
                        lhsT=ksl,
                        rhs=vsl,
                        start=True,
                        stop=True,
                    )
                et4 = etp.tile([S, CB * S], mmdt, tag="et")
                nc.scalar.activation(et4[:], ps4[:], Exp, scale=1.0 / NPIX)
                po4 = po_pool.tile([S, CB * (S + 1)], F32, tag="po")
                for i in range(CB):
                    cl = cl0 + i
                    q97 = projv[:, :, ql * JW + 1 * CL + cl]
                    nc.tensor.matmul(
                        po4[:, i * (S + 1) : (i + 1) * (S + 1)],
                        lhsT=et4[:, i * S : (i + 1) * S],
                        rhs=q97,
                        start=True,
                        stop=True,
                    )
                po4v = po4[:].rearrange("p (i w) -> p i w", w=S + 1)
                rc4 = rcp.tile([S, CB], F32, tag="rc")
                nc.vector.reciprocal(rc4[:], po4v[:, :, S])
                obv = OB[:, cl0 * S : (cl0 + CB) * S].rearrange(
                    "p (i z) -> p i z", z=S
                )
                rc4b, _ = broadcast_tensor_aps(
                    rc4[:].rearrange("p (i o) -> p i o", o=1), po4v[:, :, 0:S]
                )
                nc.vector.tensor_mul(obv, po4v[:, :, 0:S], rc4b)

            # gather: OB[x, (cl, z)] -> HFIN[rows, (x, z)] via a DRAM
            # bounce (contiguous write, then strided read with the
            # (c x z) ordering on the DRAM side). Two chunks per group
            # so the final group's gather overlaps its own attention.
            HCL = CL // 2
            for half in range(2):
                r0 = q * CL + half * HCL
                hfh, row = divmod(r0, 128)
                DQ = dramp.tile([S, HCL * S], findt, tag="dq")
                nc.sync.dma_start(
                    DQ[:], OB[:, half * HCL * S : (half + 1) * HCL * S]
                )
                nc.gpsimd.dma_start(
                    HFIN[row : row + HCL, hfh * NPIX : (hfh + 1) * NPIX],
                    DQ[:].rearrange("x (c z) -> c x z", z=S),
                )

    # to_out projection: contract all 256 (h,c) rows
    for n0 in range(0, NPIX, FCH):
        pf = pf_pool.tile([C, FCH], F32, tag="ps")
        nc.tensor.matmul(
            pf[:], lhsT=W2T[:, 0:C], rhs=HFIN[:, n0 : n0 + FCH], start=True, stop=False
        )
        nc.tensor.matmul(
            pf[:],
            lhsT=W2T[:, C : 2 * C],
            rhs=HFIN[:, NPIX + n0 : NPIX + n0 + FCH],
            start=False,
            stop=True,
        )
        fst = stp.tile([C, FCH], F32, tag="fst")
        if (n0 // FCH) % 2 == 0:
            nc.vector.tensor_copy(fst[:], pf[:])
        else:
            nc.scalar.activation(fst[:], pf[:], Copy)
        del pf
        nc.sync.dma_start(outp[:, n0 : n0 + FCH], fst[:])


_NC_CACHE = {}


def build_nc(cfg_key=None):
    cfg = CFG
    key = (cfg["mmdt"], cfg["findt"])
    if key in _NC_CACHE:
        return _NC_CACHE[key]
    nc = bacc.Bacc("TRN2", target_bir_lowering=False, debug=False)
    xe = nc.dram_tensor("xe", [C + 1, NPIX], cfg["mmdt"], kind="ExternalInput").ap()
    wtg = nc.dram_tensor(
        "wtg", [C + 1, NQ * JW], cfg["mmdt"], kind="ExternalInput"
    ).ap()
    w2t = nc.dram_tensor("w2t", [2 * 128, C], cfg["findt"], kind="ExternalInput").ap()
    outp = nc.dram_tensor("outp", [C, NPIX], F32, kind="ExternalOutput").ap()
    with tile.TileContext(nc) as tc:
        with ExitStack() as ctx:
            _body(ctx, tc, xe, wtg, w2t, outp, cfg)
    nc.compile()
    _NC_CACHE[key] = nc
    return nc


def prep_in_maps(x, w_vkq, b_vkq, w_out, b_out):
    mmnp = np.dtype(mybir.dt.np(CFG["mmdt"]))
    finp = np.dtype(mybir.dt.np(CFG["findt"]))
    x = np.asarray(x, np.float32)
    w_vkq = np.asarray(w_vkq, np.float32)
    b_vkq = np.asarray(b_vkq, np.float32)
    w_out = np.asarray(w_out, np.float32)
    in_maps = []
    for core in range(NCORES):
        b, hh = divmod(core, 2)
        xe = np.concatenate(
            [x[b].reshape(C, NPIX), np.ones((1, NPIX), np.float32)], axis=0
        )
        wtg = np.empty((C + 1, NQ * JW), np.float32)
        w2t = np.empty((256, C), np.float32)
        for qq in range(NQ):
            h = hh * HL + qq // 2
            cb = (qq % 2) * CL
            for s in range(3):
                o = s * (H * C) + h * C + cb
                j = qq * JW + s * CL
                wtg[0:C, j : j + CL] = w_vkq[o : o + CL, :].T
                wtg[C, j : j + CL] = b_vkq[o : o + CL]
            for cl in range(CL):
                w2t[qq * CL + cl, :] = w_out[:, (cb + cl) * H + h]
        in_maps.append(
            {
                "xe": xe.astype(mmnp),
                "wtg": wtg.astype(mmnp),
                "w2t": w2t.astype(finp),
            }
        )
    return in_maps


def combine(results, b_out):
    b_out = np.asarray(b_out, np.float32)
    out = np.empty((B, C, S, S), np.float32)
    for b in range(B):
        part = results[2 * b]["outp"].astype(np.float32) + results[2 * b + 1][
            "outp"
        ].astype(np.float32)
        out[b] = part.reshape(C, S, S) + b_out[:, None, None]
    return out


def kernel(x, w_vkq, b_vkq, w_out, b_out):
    nc = build_nc()
    in_maps = prep_in_maps(x, w_vkq, b_vkq, w_out, b_out)
    r = run_bass_kernel_spmd(nc, in_maps, list(range(NCORES)), trace=False)
    kernel.last_result = r
    return combine(r.results, b_out)
